# revision 26
# baseline (speedup 1.0000x reference)
"""CARAFE (content-aware reassembly of features) TRN2 Bass kernel.

Problem: input [8, 256, 64, 64], kernel_map [8, 100, 64, 64] (100 = up^2 *
k^2 with up=2, k=5), output [8, 256, 128, 128].

Strategy: data-parallel over batch across 8 NeuronCores (one image per
core). Per core the reassembly is a banded matmul contracting over 2D
windows: output columns are grouped into 8 blocks of 8 output w each; a
block's 32 output columns (8 w x 4 subpixels) read a 5-row x 12-col input
window, so each output row h is

    out[c, (w, u)] += sum_{(row, wl)} Quad[(row, wl), c] * Band[(row, wl), (w, u)]

Input rows are packed in quad tiles [(4 rows x 12 wl) = 48 partitions,
(block, c)] streamed straight from HBM in a host-prepared sliding-window
layout. The 5-row window of output row h spans exactly two quads, so each
(h, block) is 2 accumulating matmuls, both reading from partition base 0
(a hardware requirement); band rows for out-of-window quad rows are zero.
Band density is 25/78 avg (vs 5/64 for full-row banding) -- ~3x less band
DMA. Operands are fp16, PSUM accumulates fp32, pixel-shuffle happens in
the PSUM eviction copy; output is stored fp16 and upcast on the host.
"""

import numpy as np

import concourse.bass as bass
import concourse.mybir as mybir
import concourse.tile as tile
from concourse.bass_utils import run_bass_kernel_spmd

B, C, H, W = 8, 256, 64, 64
K, UP = 5, 2
PAD = K // 2
U2 = UP * UP
H2, W2 = H * UP, W * UP
FP32 = mybir.dt.float32
FP16 = mybir.dt.float16

NB = 8            # output-w blocks per row
WO = W // NB      # output w per block (8)
WL = WO + K - 1   # input cols per block window (12)
QR = 4            # rows per quad tile
QP = QR * WL      # quad partitions (48)
NQ = H // QR + 2  # quads incl leading/trailing zero-pad quads (18)
NCOL = WO * U2    # matmul output columns per block (32)
HH = 8            # output rows per group
NG = H // HH      # groups (8)
ROWF = NB * C     # quad tile free elems (2048)
B1F = HH * NB * NCOL        # chunk-1 band tile free elems per group (2048)
B2F = 4 * NB * NCOL         # chunk-2 band tile free elems per group (1024)
# chunk-2 columns grouped by k2 class: tile A rows=48 (k2 48, 36),
# tile B rows=24 (k2 24, 12); hh -> (tile, col group) with r1 = (hh+2)%4
T2_SLOT = {1: ("a", 0), 5: ("a", 1), 0: ("a", 2), 4: ("a", 3),
           3: ("b", 0), 7: ("b", 1), 2: ("b", 2), 6: ("b", 3)}
GBYTES = (QP * B1F + QP * B2F + (QP // 2) * B2F)  # band stream elems/group


def _build_bass():
    nc = bass.Bass()
    xs_d = nc.declare_dram_parameter("xstream", [1, NQ * QP * ROWF], FP16,
                                     isOutput=False)
    bd_d = nc.declare_dram_parameter("bandstream", [1, NG * GBYTES], FP16,
                                     isOutput=False)
    y_d = nc.declare_dram_parameter("y", [1, C * H2 * W2], FP16, isOutput=True)

    xs_v = xs_d[0].rearrange("(p f) -> p f", p=NQ * QP)       # [864, 2048]
    y_v = y_d[0].rearrange("(c h w) -> c h w", c=C, h=H2)     # [C, H2, W2]

    with tile.TileContext(nc) as tc:
        with (
            tc.tile_pool(name="quad", bufs=8) as quad_pool,
            tc.tile_pool(name="band", bufs=4) as band_pool,
            tc.tile_pool(name="stg", bufs=6) as stg_pool,
            tc.tile_pool(name="ps", bufs=8, space="PSUM") as ps_pool,
        ):
            qt = {}

            def load_quad(qi):
                t = quad_pool.tile([QP, ROWF], FP16, name="qt", tag="qt")
                nc.sync.dma_start(t[:], xs_v[qi * QP : (qi + 1) * QP, :])
                qt[qi] = t

            def load_band(g, split=False):
                b1 = band_pool.tile([QP, B1F], FP16, name="b1", tag="b1")
                b2a = band_pool.tile([QP, B2F], FP16, name="b2a", tag="b2a")
                b2b = band_pool.tile([QP // 2, B2F], FP16, name="b2b", tag="b2b")
                o = g * GBYTES
                v1 = bd_d[0, o : o + QP * B1F].rearrange("(p f) -> p f", p=QP)
                o += QP * B1F
                v2a = bd_d[0, o : o + QP * B2F].rearrange("(p f) -> p f", p=QP)
                o += QP * B2F
                v2b = bd_d[0, o : o + (QP // 2) * B2F].rearrange(
                    "(p f) -> p f", p=QP // 2
                )
                if split:
                    q = B1F // 2
                    nc.scalar.dma_start(b1[:, 0:q], v1[:, 0:q])
                    nc.scalar.dma_start(b2a[:], v2a)
                    nc.gpsimd.dma_start(b1[:, q:B1F], v1[:, q:B1F])
                    nc.gpsimd.dma_start(b2b[:], v2b)
                else:
                    nc.gpsimd.dma_start(b1[:], v1)
                    nc.gpsimd.dma_start(b2a[:], v2a)
                    nc.gpsimd.dma_start(b2b[:], v2b)
                return b1, b2a, b2b

            bts = [load_band(0, split=True)]
            for qi in range(1, 4):
                load_quad(qi)
            bts.append(load_band(1))
            bts.append(load_band(2))
            bts.append(load_band(3))

            stg = None
            for g in range(NG):
                b1, b2a, b2b = bts[g]
                for hh in range(HH):
                    h = g * HH + hh
                    if h % QR == 0 and h // QR + 4 < NQ - 1:
                        load_quad(h // QR + 4)
                    if h % 4 == 0:
                        stg = [
                            stg_pool.tile([128, 4, UP, W, UP], FP16,
                                          name=f"stg{ch}", tag=f"stg{ch}")
                            for ch in range(2)
                        ]
                    q0 = (h - 2) >> 2          # first quad (tile index q0+1)
                    r1 = (h - 2) & 3           # row of q0 holding di=0
                    k2 = WL * (r1 + 1)         # partitions used in quad q0+1
                    c1 = q0 + 1 > 0            # quad q0 has real rows
                    c2 = q0 + 2 < NQ - 1       # quad q0+1 has real rows
                    t2, slot = T2_SLOT[hh]
                    b2 = b2a if t2 == "a" else b2b
                    for ch in range(2):
                        ps = ps_pool.tile([128, W, UP, UP], FP32)
                        for b in range(NB):
                            fo = b * C + ch * 128
                            co = (hh * NB + b) * NCOL
                            c2o = (slot * NB + b) * NCOL
                            if c1:
                                nc.tensor.matmul(
                                    ps[:, b * WO : (b + 1) * WO, :, :],
                                    qt[q0 + 1][:, fo : fo + 128],
                                    b1[:, co : co + NCOL],
                                    start=True,
                                    stop=not c2,
                                )
                            if c2:
                                nc.tensor.matmul(
                                    ps[:, b * WO : (b + 1) * WO, :, :],
                                    qt[q0 + 2][0:k2, fo : fo + 128],
                                    b2[0:k2, c2o : c2o + NCOL],
                                    start=not c1,
                                    stop=True,
                                )
                        src = ps[:, :, :, :].transpose([0, 2, 1, 3])
                        dst = stg[ch][:, hh % 4, :, :, :]
                        if ch == 0:
                            nc.vector.tensor_copy(dst, src)
                        else:
                            nc.scalar.copy(dst, src)
                    last = g >= NG - 2
                    if (h % 4 == 3) if not last else (h % 2 == 1):
                        n = 4 if not last else 2
                        h0 = h - n + 1
                        for ch, eng in ((0, nc.sync), (1, nc.scalar)):
                            eng.dma_start(
                                y_v[ch * 128 : ch * 128 + 128,
                                    UP * h0 : UP * (h + 1), :],
                                stg[ch][:, h0 % 4 : h0 % 4 + n, :, :, :],
                            )
                if g + 4 < NG:
                    bts.append(load_band(g + 4))
    _split_overfull_waits(nc)
    return nc


def _split_overfull_waits(nc):
    """Walrus caps sem-waits per instruction (1; 2 for EventSemaphore).
    Hoist excess waits onto inserted wait-only instructions."""
    n_new = 0
    for bb in nc.main_func.blocks:
        out, changed = [], False
        for ins in bb.instructions:
            si = ins.sync_info
            waits = list(si.on_wait) if (si is not None and si.on_wait) else []
            cap = 2 if isinstance(ins, mybir.InstEventSemaphore) else 1
            if len(waits) > cap:
                keep, extra = waits[-cap:], waits[:-cap]
                while extra:
                    chunk, extra = extra[:2], extra[2:]
                    n_new += 1
                    ev = mybir.InstEventSemaphore(
                        name=f"I-waitfix-{n_new}",
                        engine=ins.engine,
                        sync_info=mybir.SyncInfo(on_wait=chunk, on_update=[]),
                        ins=[],
                        outs=[],
                    )
                    nc.register_instruction(ev)
                    out.append(ev)
                ins.sync_info = mybir.SyncInfo(
                    on_wait=keep,
                    on_update=list(si.on_update) if si.on_update else [],
                )
                changed = True
            out.append(ins)
        if changed:
            bb.instructions = out
    return n_new


def _host_xstream(x_b):
    """x_b: [C, H, W] fp32 -> [1, NQ*QP*ROWF] fp16 quad sliding-window
    stream: [quad, row, wl, b, c] with quad -1 and 16 all-zero."""
    xt = np.zeros((H + 2 * QR, W + 2 * PAD + 2, C), np.float16)
    xt[QR : QR + H, PAD : PAD + W, :] = x_b.transpose(1, 2, 0).astype(np.float16)
    s0, s1, s2 = xt.strides
    sw = np.lib.stride_tricks.as_strided(
        xt, (NQ, QR, NB, WL, C), (QR * s0, s0, WO * s1, s1, s2)
    )
    stream = sw.transpose(0, 1, 3, 2, 4)       # [quad, row, wl, b, c]
    return np.ascontiguousarray(stream).reshape(1, -1)


def _host_bandstream(km_b):
    """km_b: [100, H, W] -> [1, NG*GBYTES] fp16 band stream. Per group:
    chunk-1 tile [QP, (hh, b, wo, u)], then chunk-2 tiles A [QP, (slot,
    b, wo, u)] (k2 48/36 classes) and B [QP/2, ...] (k2 24/12 classes)."""
    km_r = km_b.reshape(U2, K, K, H, W).astype(np.float16)  # [u, di, dj, h, w]
    # [h, chunk, row, wl, b, wo, u]
    arr = np.zeros((H, 2, QR, WL, NB, WO, U2), np.float16)
    for r1 in range(QR):
        hs = np.arange(H)[(np.arange(H) - 2) % QR == r1]
        for di in range(K):
            j = r1 + di
            ck, jj = (0, j) if j < QR else (1, j - QR)
            for dj in range(K):
                # t: [h_sel, b, wo, u]
                t = km_r[:, di, dj, hs, :].reshape(U2, len(hs), NB, WO)
                t = t.transpose(1, 2, 3, 0)
                for wo in range(WO):
                    arr[hs, ck, jj, wo + dj, :, wo, :] = t[:, :, wo, :]
    arr = arr.reshape(NG, HH, 2, QP, NB * WO * U2)  # [g, hh, ck, p, f]
    parts = []
    for g in range(NG):
        # chunk-1: [QP, (hh, b, wo, u)]
        parts.append(arr[g, :, 0].transpose(1, 0, 2).reshape(-1))
        for rows, hhs in ((QP, [1, 5, 0, 4]), (QP // 2, [3, 7, 2, 6])):
            t2 = arr[g, hhs, 1][:, :rows]       # [slot, rows, f]
            parts.append(t2.transpose(1, 0, 2).reshape(-1))
    return np.ascontiguousarray(np.concatenate(parts)).reshape(1, -1)


_NC_CACHE = None


def _get_nc():
    global _NC_CACHE
    if _NC_CACHE is None:
        _NC_CACHE = _build_bass()
    return _NC_CACHE


def _prep_inputs(input, kernel_map):
    in_maps = []
    for b in range(B):
        in_maps.append({
            "xstream": _host_xstream(np.asarray(input[b])),
            "bandstream": _host_bandstream(np.asarray(kernel_map[b])),
        })
    return in_maps


def _run(input, kernel_map, trace=False):
    nc = _get_nc()
    in_maps = _prep_inputs(input, kernel_map)
    res = run_bass_kernel_spmd(nc, in_maps, list(range(B)), trace=trace)
    out = np.stack(
        [res.results[b]["y"].reshape(C, H2, W2).astype(np.float32) for b in range(B)],
        axis=0,
    )
    return out, res


def kernel(input, kernel_map):
    out, _ = _run(input, kernel_map, trace=False)
    return out


# revision 27
# speedup vs baseline: 1.0858x; 1.0858x over previous
"""CARAFE (content-aware reassembly of features) TRN2 Bass kernel.

Problem: input [8, 256, 64, 64], kernel_map [8, 100, 64, 64] (100 = up^2 *
k^2 with up=2, k=5), output [8, 256, 128, 128].

Strategy: data-parallel over batch across 8 NeuronCores (one image per
core). Per core the reassembly is a banded matmul contracting over 2D
windows: output columns are grouped into 8 blocks of 8 output w each; a
block's 32 output columns (8 w x 4 subpixels) read a 5-row x 12-col input
window, so each output row h is

    out[c, (w, u)] += sum_{(row, wl)} Quad[(row, wl), c] * Band[(row, wl), (w, u)]

Input rows are packed in quad tiles [(4 rows x 12 wl) = 48 partitions,
(block, c)] streamed straight from HBM in a host-prepared sliding-window
layout. The 5-row window of output row h spans exactly two quads, so each
(h, block) is 2 accumulating matmuls, both reading from partition base 0
(a hardware requirement); band rows for out-of-window quad rows are zero.
Band density is 25/78 avg (vs 5/64 for full-row banding) -- ~3x less band
DMA. Operands are fp16, PSUM accumulates fp32, pixel-shuffle happens in
the PSUM eviction copy; output is stored fp16 and upcast on the host.
"""

import numpy as np

import concourse.bass as bass
import concourse.mybir as mybir
import concourse.tile as tile
from concourse.bass_utils import run_bass_kernel_spmd

B, C, H, W = 8, 256, 64, 64
K, UP = 5, 2
PAD = K // 2
U2 = UP * UP
H2, W2 = H * UP, W * UP
FP32 = mybir.dt.float32
FP16 = mybir.dt.float16

NB = 8            # output-w blocks per row
WO = W // NB      # output w per block (8)
WL = WO + K - 1   # input cols per block window (12)
QR = 4            # rows per quad tile
QP = QR * WL      # quad partitions (48)
NQ = H // QR + 2  # quads incl leading/trailing zero-pad quads (18)
NCOL = WO * U2    # matmul output columns per block (32)
HH = 8            # output rows per group
NG = H // HH      # groups (8)
ROWF = NB * C     # quad tile free elems (2048)
B1F = HH * NB * NCOL        # chunk-1 band tile free elems per group (2048)
B2F = 4 * NB * NCOL         # chunk-2 band tile free elems per group (1024)
# chunk-2 columns grouped by k2 class: tile A rows=48 (k2 48, 36),
# tile B rows=24 (k2 24, 12); hh -> (tile, col group) with r1 = (hh+2)%4
T2_SLOT = {1: ("a", 0), 5: ("a", 1), 0: ("a", 2), 4: ("a", 3),
           3: ("b", 0), 7: ("b", 1), 2: ("b", 2), 6: ("b", 3)}
GBYTES = (QP * B1F + QP * B2F + (QP // 2) * B2F)  # band stream elems/group


def _build_bass():
    nc = bass.Bass()
    xs_d = nc.declare_dram_parameter("xstream", [1, NQ * QP * ROWF], FP16,
                                     isOutput=False)
    bd_d = nc.declare_dram_parameter("bandstream", [1, NG * GBYTES], FP16,
                                     isOutput=False)
    y_d = nc.declare_dram_parameter("y", [1, C * H2 * W2], FP16, isOutput=True)

    xs_v = xs_d[0].rearrange("(p f) -> p f", p=NQ * QP)       # [864, 2048]
    y_v = y_d[0].rearrange("(c h w) -> c h w", c=C, h=H2)     # [C, H2, W2]

    with tile.TileContext(nc) as tc:
        with (
            tc.tile_pool(name="quad", bufs=5) as quad_pool,
            tc.tile_pool(name="band", bufs=3) as band_pool,
            tc.tile_pool(name="stg", bufs=6) as stg_pool,
            tc.tile_pool(name="ps", bufs=8, space="PSUM") as ps_pool,
        ):
            qt = {}

            def load_quad(qi):
                t = quad_pool.tile([QP, ROWF], FP16, name="qt", tag="qt")
                nc.sync.dma_start(t[:], xs_v[qi * QP : (qi + 1) * QP, :])
                qt[qi] = t

            def load_band(g, split=False):
                b1 = band_pool.tile([QP, B1F], FP16, name="b1", tag="b1")
                b2a = band_pool.tile([QP, B2F], FP16, name="b2a", tag="b2a")
                b2b = band_pool.tile([QP // 2, B2F], FP16, name="b2b", tag="b2b")
                o = g * GBYTES
                v1 = bd_d[0, o : o + QP * B1F].rearrange("(p f) -> p f", p=QP)
                o += QP * B1F
                v2a = bd_d[0, o : o + QP * B2F].rearrange("(p f) -> p f", p=QP)
                o += QP * B2F
                v2b = bd_d[0, o : o + (QP // 2) * B2F].rearrange(
                    "(p f) -> p f", p=QP // 2
                )
                if split:
                    q = B1F // 2
                    nc.scalar.dma_start(b1[:, 0:q], v1[:, 0:q])
                    nc.scalar.dma_start(b2a[:], v2a)
                    nc.gpsimd.dma_start(b1[:, q:B1F], v1[:, q:B1F])
                    nc.gpsimd.dma_start(b2b[:], v2b)
                else:
                    nc.gpsimd.dma_start(b1[:], v1)
                    nc.gpsimd.dma_start(b2a[:], v2a)
                    nc.gpsimd.dma_start(b2b[:], v2b)
                return b1, b2a, b2b

            bts = [load_band(0, split=True)]
            for qi in range(1, 4):
                load_quad(qi)
            bts.append(load_band(1))
            bts.append(load_band(2))

            stg = None
            for g in range(NG):
                b1, b2a, b2b = bts[g]
                for hh in range(HH):
                    h = g * HH + hh
                    if h % QR == 0 and h // QR + 4 < NQ - 1:
                        load_quad(h // QR + 4)
                    if h % 4 == 0:
                        stg = [
                            stg_pool.tile([128, 4, UP, W, UP], FP16,
                                          name=f"stg{ch}", tag=f"stg{ch}")
                            for ch in range(2)
                        ]
                    q0 = (h - 2) >> 2          # first quad (tile index q0+1)
                    r1 = (h - 2) & 3           # row of q0 holding di=0
                    k2 = WL * (r1 + 1)         # partitions used in quad q0+1
                    c1 = q0 + 1 > 0            # quad q0 has real rows
                    c2 = q0 + 2 < NQ - 1       # quad q0+1 has real rows
                    t2, slot = T2_SLOT[hh]
                    b2 = b2a if t2 == "a" else b2b
                    for ch in range(2):
                        ps = ps_pool.tile([128, W, UP, UP], FP32)
                        for b in range(NB):
                            fo = b * C + ch * 128
                            co = (hh * NB + b) * NCOL
                            c2o = (slot * NB + b) * NCOL
                            if c1:
                                nc.tensor.matmul(
                                    ps[:, b * WO : (b + 1) * WO, :, :],
                                    qt[q0 + 1][:, fo : fo + 128],
                                    b1[:, co : co + NCOL],
                                    start=True,
                                    stop=not c2,
                                )
                            if c2:
                                nc.tensor.matmul(
                                    ps[:, b * WO : (b + 1) * WO, :, :],
                                    qt[q0 + 2][0:k2, fo : fo + 128],
                                    b2[0:k2, c2o : c2o + NCOL],
                                    start=not c1,
                                    stop=True,
                                )
                        src = ps[:, :, :, :].transpose([0, 2, 1, 3])
                        dst = stg[ch][:, hh % 4, :, :, :]
                        if ch == 0:
                            nc.vector.tensor_copy(dst, src)
                        else:
                            nc.scalar.copy(dst, src)
                    last = g == NG - 1
                    if (h % 4 == 3) if not last else (h % 2 == 1):
                        n = 4 if not last else 2
                        h0 = h - n + 1
                        for ch, eng in ((0, nc.sync), (1, nc.scalar)):
                            eng.dma_start(
                                y_v[ch * 128 : ch * 128 + 128,
                                    UP * h0 : UP * (h + 1), :],
                                stg[ch][:, h0 % 4 : h0 % 4 + n, :, :, :],
                            )
                if g + 3 < NG:
                    bts.append(load_band(g + 3))
    _split_overfull_waits(nc)
    return nc


def _split_overfull_waits(nc):
    """Walrus caps sem-waits per instruction (1; 2 for EventSemaphore).
    Hoist excess waits onto inserted wait-only instructions."""
    n_new = 0
    for bb in nc.main_func.blocks:
        out, changed = [], False
        for ins in bb.instructions:
            si = ins.sync_info
            waits = list(si.on_wait) if (si is not None and si.on_wait) else []
            cap = 2 if isinstance(ins, mybir.InstEventSemaphore) else 1
            if len(waits) > cap:
                keep, extra = waits[-cap:], waits[:-cap]
                while extra:
                    chunk, extra = extra[:2], extra[2:]
                    n_new += 1
                    ev = mybir.InstEventSemaphore(
                        name=f"I-waitfix-{n_new}",
                        engine=ins.engine,
                        sync_info=mybir.SyncInfo(on_wait=chunk, on_update=[]),
                        ins=[],
                        outs=[],
                    )
                    nc.register_instruction(ev)
                    out.append(ev)
                ins.sync_info = mybir.SyncInfo(
                    on_wait=keep,
                    on_update=list(si.on_update) if si.on_update else [],
                )
                changed = True
            out.append(ins)
        if changed:
            bb.instructions = out
    return n_new


def _host_xstream(x_b):
    """x_b: [C, H, W] fp32 -> [1, NQ*QP*ROWF] fp16 quad sliding-window
    stream: [quad, row, wl, b, c] with quad -1 and 16 all-zero."""
    xt = np.zeros((H + 2 * QR, W + 2 * PAD + 2, C), np.float16)
    xt[QR : QR + H, PAD : PAD + W, :] = x_b.transpose(1, 2, 0).astype(np.float16)
    s0, s1, s2 = xt.strides
    sw = np.lib.stride_tricks.as_strided(
        xt, (NQ, QR, NB, WL, C), (QR * s0, s0, WO * s1, s1, s2)
    )
    stream = sw.transpose(0, 1, 3, 2, 4)       # [quad, row, wl, b, c]
    return np.ascontiguousarray(stream).reshape(1, -1)


def _host_bandstream(km_b):
    """km_b: [100, H, W] -> [1, NG*GBYTES] fp16 band stream. Per group:
    chunk-1 tile [QP, (hh, b, wo, u)], then chunk-2 tiles A [QP, (slot,
    b, wo, u)] (k2 48/36 classes) and B [QP/2, ...] (k2 24/12 classes)."""
    km_r = km_b.reshape(U2, K, K, H, W).astype(np.float16)  # [u, di, dj, h, w]
    # [h, chunk, row, wl, b, wo, u]
    arr = np.zeros((H, 2, QR, WL, NB, WO, U2), np.float16)
    for r1 in range(QR):
        hs = np.arange(H)[(np.arange(H) - 2) % QR == r1]
        for di in range(K):
            j = r1 + di
            ck, jj = (0, j) if j < QR else (1, j - QR)
            for dj in range(K):
                # t: [h_sel, b, wo, u]
                t = km_r[:, di, dj, hs, :].reshape(U2, len(hs), NB, WO)
                t = t.transpose(1, 2, 3, 0)
                for wo in range(WO):
                    arr[hs, ck, jj, wo + dj, :, wo, :] = t[:, :, wo, :]
    arr = arr.reshape(NG, HH, 2, QP, NB * WO * U2)  # [g, hh, ck, p, f]
    parts = []
    for g in range(NG):
        # chunk-1: [QP, (hh, b, wo, u)]
        parts.append(arr[g, :, 0].transpose(1, 0, 2).reshape(-1))
        for rows, hhs in ((QP, [1, 5, 0, 4]), (QP // 2, [3, 7, 2, 6])):
            t2 = arr[g, hhs, 1][:, :rows]       # [slot, rows, f]
            parts.append(t2.transpose(1, 0, 2).reshape(-1))
    return np.ascontiguousarray(np.concatenate(parts)).reshape(1, -1)


_NC_CACHE = None


def _get_nc():
    global _NC_CACHE
    if _NC_CACHE is None:
        _NC_CACHE = _build_bass()
    return _NC_CACHE


def _prep_inputs(input, kernel_map):
    in_maps = []
    for b in range(B):
        in_maps.append({
            "xstream": _host_xstream(np.asarray(input[b])),
            "bandstream": _host_bandstream(np.asarray(kernel_map[b])),
        })
    return in_maps


def _run(input, kernel_map, trace=False):
    nc = _get_nc()
    in_maps = _prep_inputs(input, kernel_map)
    res = run_bass_kernel_spmd(nc, in_maps, list(range(B)), trace=trace)
    out = np.stack(
        [res.results[b]["y"].reshape(C, H2, W2).astype(np.float32) for b in range(B)],
        axis=0,
    )
    return out, res


def kernel(input, kernel_map):
    out, _ = _run(input, kernel_map, trace=False)
    return out
